# revision 1
# baseline (speedup 1.0000x reference)
"""TRN2 Bass kernel for nn_CombCrossAttention (GQA cross-attention block).

Computation (T=2048, K=2048, E=4096, H=32 q-heads, KVH=8 kv-heads, D=128):
    q  = hidden @ Wq.T;  per-head RMSNorm(q) * q_norm_w
    kn = RMSNorm(k) * k_norm_w  (GQA: each kv head serves 4 q heads)
    attn = softmax(qn @ kn.T / sqrt(D)) @ v
    out  = attn @ Wo.T

Sharding: tensor-parallel over heads on 8 NeuronCores. Core c owns q-heads
4c..4c+3 (Wq rows 512c..512c+512) and kv-head c, plus Wo columns
512c..512c+512; each core emits a [T, E] partial of the o-projection (bf16)
and the host sums the 8 partials (the "all-reduce").

Everything is computed transposed ([feature, t]) so no on-chip transposes
are needed. All matmul inputs are bf16 (same PE row rate as f32r, half the
DMA/SBUF footprint) except the q-RMS square-sum path which stays f32r.
Matmul outputs are per-PSUM-bank [128,512] (ISA limit), but consecutive
matmuls share the same stationary lhsT wherever possible, and exp/square/
ln read multi-bank PSUM regions in one activation instruction to amortize
per-instruction overhead on the scalar engine.

The softmax denominator is NOT computed with per-kk ones-matmuls: exp
tiles are accumulated elementwise on the DVE (bf16) and one ones-matmul
per head does the final cross-partition sum; 1/denom uses the fast
approximate reciprocal (18 bits, ~5x faster than vector.reciprocal).
exp() needs no max-subtraction: post-RMSNorm scores are O(1)-bounded.
"""
import sys

sys.path.insert(0, "/opt/trn_rl_repo")

import numpy as np
import ml_dtypes

import jax
try:
    jax.config.update("jax_compilation_cache_dir", "/tmp/jax_neff_cache")
    jax.config.update("jax_persistent_cache_min_compile_time_secs", 1.0)
except Exception:
    pass

import concourse.bass as bass  # noqa: F401
import concourse.mybir as mybir
import concourse.tile as tile
from concourse import bacc, bass_utils

EPS = 1e-5
T, K, E, H, KVH, D = 2048, 2048, 4096, 32, 8, 128
N_CORES = 8
HL = H // N_CORES      # 4 q-heads per core
EL = HL * D            # 512 local embed rows/cols
f32 = mybir.dt.float32
f32r = mybir.dt.float32r
bf16 = mybir.dt.bfloat16

Ln = mybir.ActivationFunctionType.Ln
Exp = mybir.ActivationFunctionType.Exp


def _kernel_body(tc):
    nc = tc.nc
    hid = nc.dram_tensor("hid", [32, 128, 2048], bf16, kind="ExternalInput").ap()
    wq = nc.dram_tensor("wq", [4, 128, 32, 128], bf16, kind="ExternalInput").ap()
    kpp = nc.dram_tensor("kpp", [128, 16, 128], bf16, kind="ExternalInput").ap()
    vt = nc.dram_tensor("vt", [128, 16, 128], bf16, kind="ExternalInput").ap()
    wo = nc.dram_tensor("wo", [4, 128, 32, 128], bf16, kind="ExternalInput").ap()
    onesf = nc.dram_tensor("onesf", [128, 128], f32r, kind="ExternalInput").ap()
    onesb = nc.dram_tensor("onesb", [128, 128], bf16, kind="ExternalInput").ap()
    outp = nc.dram_tensor("outp", [32, 128, 4, 512], bf16, kind="ExternalOutput").ap()

    with tc.tile_pool(name="persist", bufs=1) as persist:
        ones = persist.tile([128, 128], f32r)
        nc.gpsimd.dma_start(out=ones, in_=onesf)
        ones_b = persist.tile([128, 128], bf16)
        nc.gpsimd.dma_start(out=ones_b, in_=onesb)
        eps_col = persist.tile([128, 1], f32)
        nc.vector.memset(eps_col, EPS)
        k_sb = persist.tile([128, 16, 128], bf16)
        nc.gpsimd.dma_start(out=k_sb, in_=kpp)
        v_sb = persist.tile([128, 16, 128], bf16)
        nc.gpsimd.dma_start(out=v_sb, in_=vt)
        qT = persist.tile([128, HL, 4, 512], bf16)  # [D, head, tcn, t]

        # ---- Phase 1: q-proj + fused per-head RMSNorm scale ----
        # t is processed in halves nh (1024 cols); per (nh, m) the k-loop
        # does 2 consecutive same-lhsT matmuls (one per 512-col PSUM bank).
        with tc.tile_pool(name="hidp", bufs=1) as hidp, \
             tc.tile_pool(name="wqp", bufs=2) as wqp, \
             tc.tile_pool(name="sqp", bufs=1) as sqp, \
             tc.tile_pool(name="srp", bufs=1) as srp, \
             tc.tile_pool(name="qps", bufs=2, space="PSUM") as qps, \
             tc.tile_pool(name="sps", bufs=2, space="PSUM") as sps:
            hid_sb = hidp.tile([128, 32, 4, 512], bf16)
            for k in range(32):
                nc.sync.dma_start(out=hid_sb[:, k], in_=hid[k])
            for nh in range(2):
                for m in range(HL):
                    wq_m = wqp.tile([128, 32, 128], bf16, tag="wqm",
                                    name=f"wq{m}")
                    nc.scalar.dma_start(out=wq_m, in_=wq[m])
                    pq = qps.tile([128, 2, 512], f32, tag="pq")
                    for k in range(32):
                        for j in range(2):
                            nc.tensor.matmul(pq[:, j, :], wq_m[:, k, :],
                                             hid_sb[:, k, 2 * nh + j, :],
                                             start=(k == 0), stop=(k == 31))
                    # RMSNorm scale: qT = pq * rsqrt(mean_d(pq^2) + eps),
                    # rsqrt = exp(-0.5*ln(.)); cross-partition sum of squares
                    # via two consecutive same-lhsT ones-matmuls.
                    sq = sqp.tile([128, 2, 512], f32r, tag="sq")
                    nc.scalar.square(sq, pq)
                    ps = sps.tile([128, 2, 512], f32, tag="ps")
                    for j in range(2):
                        nc.tensor.matmul(ps[:, j, :], ones, sq[:, j, :],
                                         start=True, stop=True)
                    lns = srp.tile([128, 2, 512], f32, tag="ln")
                    nc.scalar.activation(lns, ps, Ln, scale=1.0 / D,
                                         bias=eps_col[:])
                    rinv = srp.tile([128, 2, 512], f32, tag="ri")
                    nc.scalar.activation(rinv, lns, Exp, scale=-0.5)
                    nc.vector.tensor_mul(
                        qT[:, m, 2 * nh:2 * nh + 2, :], pq, rinv)

        # ---- Phases 2+3 ----
        with tc.tile_pool(name="wop", bufs=1) as wop, \
             tc.tile_pool(name="aop", bufs=1) as aop:
            wo_sb = wop.tile([128, 4, 32, 128], bf16)
            for k in range(4):
                nc.scalar.dma_start(out=wo_sb[:, k], in_=wo[k])
            aoT = aop.tile([128, HL, 2048], bf16)  # attn_out.T, local heads

            # Phase 2: attention. Per kk: 4 consecutive scores matmuls share
            # the k'' lhsT (2 PSUM tiles x 2 head slices), 2 batched exps,
            # 4 consecutive v-matmuls share the v lhsT, DVE accumulates the
            # denominator in bf16.
            with tc.tile_pool(name="expp", bufs=3) as expp, \
                 tc.tile_pool(name="accp", bufs=2) as accp, \
                 tc.tile_pool(name="rdp", bufs=2) as rdp, \
                 tc.tile_pool(name="scps", bufs=2, space="PSUM") as scps, \
                 tc.tile_pool(name="ops", bufs=4, space="PSUM") as ops:
                for tcn in range(4):
                    ts = slice(tcn * 512, (tcn + 1) * 512)
                    po = [ops.tile([128, 512], f32, tag="po", name=f"po{h}")
                          for h in range(HL)]
                    exacc = [accp.tile([128, 2, 512], bf16, tag="ea",
                                       name=f"ea{g}") for g in range(2)]
                    for kk in range(16):
                        pscr = [scps.tile([128, 2, 512], f32, tag="sc",
                                          name=f"sc{g}") for g in range(2)]
                        for g in range(2):
                            for j in range(2):
                                nc.tensor.matmul(pscr[g][:, j, :],
                                                 k_sb[:, kk, :],
                                                 qT[:, 2 * g + j, tcn, :],
                                                 start=True, stop=True)
                        exs = []
                        for g in range(2):
                            ex = expp.tile([128, 2, 512], bf16, tag="ex",
                                           name=f"ex{g}")
                            nc.scalar.activation(ex, pscr[g], Exp)
                            exs.append(ex)
                        for g in range(2):
                            for j in range(2):
                                nc.tensor.matmul(po[2 * g + j],
                                                 v_sb[:, kk, :],
                                                 exs[g][:, j, :],
                                                 start=(kk == 0),
                                                 stop=(kk == 15))
                        for g in range(2):
                            if kk == 0:
                                nc.vector.tensor_copy(exacc[g], exs[g])
                            else:
                                nc.vector.tensor_add(exacc[g], exacc[g],
                                                     exs[g])
                    for g in range(2):
                        pd = scps.tile([128, 2, 512], f32, tag="sc",
                                       name=f"pd{g}")
                        for j in range(2):
                            nc.tensor.matmul(pd[:, j, :], ones_b,
                                             exacc[g][:, j, :],
                                             start=True, stop=True)
                        rd = rdp.tile([128, 2, 512], f32, tag="rd")
                        nc.vector.reciprocal_approx_fast(out=rd, in_=pd)
                        for j in range(2):
                            nc.vector.tensor_mul(aoT[:, 2 * g + j, ts],
                                                 po[2 * g + j], rd[:, j, :])

            # Phase 3: o-projection partial  outT[j, t] = Wo_shard.T @ aoT
            with tc.tile_pool(name="obp", bufs=4) as obp, \
                 tc.tile_pool(name="pop", bufs=2, space="PSUM") as pop:
                drains = [nc.scalar.copy, nc.vector.tensor_copy]
                dmas = [nc.sync, nc.scalar]
                for m in range(32):
                    pout = pop.tile([128, 4, 512], f32, tag="pp")
                    for k in range(4):
                        for j in range(4):
                            nc.tensor.matmul(pout[:, j, :], wo_sb[:, k, m, :],
                                             aoT[:, k, 512 * j:512 * (j + 1)],
                                             start=(k == 0), stop=(k == 3))
                    ob = obp.tile([128, 4, 512], bf16, tag="ob")
                    drains[m % 2](ob, pout)
                    dmas[m % 2].dma_start(out=outp[m], in_=ob)


_NC_CACHE = None


def _build():
    global _NC_CACHE
    if _NC_CACHE is None:
        nc = bacc.Bacc("TRN2", target_bir_lowering=False, debug=False,
                       num_devices=N_CORES)
        with tile.TileContext(nc) as tc:
            _kernel_body(tc)
        nc.compile()
        _NC_CACHE = nc
    return _NC_CACHE


def _prepare_in_maps(hidden_states, k, v, Wq, Wo, q_norm_w, k_norm_w):
    bf = ml_dtypes.bfloat16
    hs = np.asarray(hidden_states, np.float32)
    k_ = np.asarray(k, np.float32)[0]      # [K, KVH, D]
    v_ = np.asarray(v, np.float32)[0]
    Wq_ = np.asarray(Wq, np.float32)
    Wo_ = np.asarray(Wo, np.float32)
    wqn = np.asarray(q_norm_w, np.float64)
    wkn = np.asarray(k_norm_w, np.float64)

    # Fold k-RMSNorm, both norm weights, and the attention scale into k''.
    kd = k_.astype(np.float64)
    rk = 1.0 / np.sqrt((kd ** 2).mean(-1, keepdims=True) + EPS)
    kpp_full = (kd * rk * (wqn * wkn) * (D ** -0.5)).astype(np.float32)

    hidT = np.ascontiguousarray(hs.T)                                  # [E, T]
    hid_tiles = np.ascontiguousarray(
        hidT.reshape(32, 128, 2048).astype(bf))
    onesf_arr = np.ones((128, 128), np.float32)
    onesb_arr = np.ones((128, 128), bf)

    in_maps = []
    for c in range(N_CORES):
        wqT = np.ascontiguousarray(Wq_[c * EL:(c + 1) * EL, :].T)      # [E, EL]
        # [m, p, k, col]: lhsT tiles for head m, partition-major like SBUF
        wq_tiles = np.ascontiguousarray(
            wqT.reshape(32, 128, 4, 128).transpose(2, 1, 0, 3).astype(bf))
        woT = np.ascontiguousarray(Wo_[:, c * EL:(c + 1) * EL].T)      # [EL, E]
        wo_tiles = np.ascontiguousarray(
            woT.reshape(4, 128, 32, 128).astype(bf))                   # [k,p,m,c]
        kppT = np.ascontiguousarray(kpp_full[:, c, :].T)               # [D, K]
        kpp_tiles = np.ascontiguousarray(
            kppT.reshape(128, 16, 128).astype(bf))
        v_tiles = np.ascontiguousarray(
            v_[:, c, :].reshape(16, 128, 128).transpose(1, 0, 2).astype(bf))
        in_maps.append({
            "hid": hid_tiles, "wq": wq_tiles, "kpp": kpp_tiles,
            "vt": v_tiles, "wo": wo_tiles,
            "onesf": onesf_arr, "onesb": onesb_arr,
        })
    return in_maps


def _gather(results):
    total = np.zeros((E, T), np.float32)
    for r in results:
        total += r["outp"].astype(np.float32).reshape(E, T)
    return np.ascontiguousarray(total.T)


def kernel(hidden_states, k, v, Wq, Wo, q_norm_w, k_norm_w):
    nc = _build()
    in_maps = _prepare_in_maps(hidden_states, k, v, Wq, Wo, q_norm_w, k_norm_w)
    res = bass_utils.run_bass_kernel_spmd(nc, in_maps,
                                          core_ids=list(range(N_CORES)))
    return _gather(res.results)

